# revision 1
# baseline (speedup 1.0000x reference)
"""Trainium2 Bass kernel for nn_RelationalAGG (2-etype GNN attention message passing).

Design: dst-sharded across 8 cores, zero collectives, zero indirect DMA.

 - Host packs dst nodes into (<=128-node, <=640-edge) tiles via balanced
   bin-packing, so every tile needs exactly ch=5 edge chunks (vs 6-7 with
   contiguous range sharding) and all cores carry equal load.
 - Host gathers per-edge src features into two fp16 stream layouts:
   normal [edge, 257] (payload + ones column) for the PE scatter matmul,
   and transposed [feat, edge] for the PE score matmuls. Dst-node feature
   tables are shipped pre-transposed (plain contiguous DMA, no DMA
   transpose on device).
 - Device, per 128-dst tile:
     gT window = WT_eff^T-block matmuls vs pre-transposed dst features
     scores    S' [e, 5*128] = fT-block matmuls into ONE grouped PSUM
               region (all-pairs vs the tile's 128 dst columns)
     E' = exp(S') -- ONE batched ACT op per tile
     masks (one-hot of dst col) on Pool; W = E'*mask as ONE DVE op
     PSUM [P, 257] += W_k^T @ [f | 1]  (PE segment-sum: messages+att sums)
   then normalize+relu (ACT, accum_out stats), variance via DVE
   tensor_tensor_reduce, layernorm applied via DVE tensor_scalar, with
   sqrt batched KFIN=14 tiles to amortize ACT table loads. Output fp16.
 - Engine balance (modeled): DMA ~2.2us/tile (bottleneck), ACT ~1.3,
   DVE ~1.1, PE ~1.3, Pool ~0.8.
"""

import math
import sys

import numpy as np

for _p in ("/opt/trn_rl_repo",):
    if _p not in sys.path:
        sys.path.insert(0, _p)

# Problem constants (hardcoded per the task contract).
NA = 100000
NB = 100000
E = 500000
D = 256
EPS = 1e-5
N_CORES = 8
P = 128
NT = 98              # tiles per core per etype
NT_G = NT * N_CORES  # global tiles per etype

TRACE = False
LAST_RESULT = None


# --------------------------------------------------------------------------
# Host-side preprocessing
# --------------------------------------------------------------------------

def _pack_tiles(deg, n_tiles, cap_nodes=P, cap_edges=5 * P):
    """Balanced bin-packing: nodes -> tiles s.t. each tile has <=cap_nodes
    nodes and (usually) <=cap_edges edges. Greedy best-fit by descending
    degree, vectorized per degree class."""
    n_nodes = len(deg)
    tile_of = np.full(n_nodes, -1, np.int32)
    col_of = np.full(n_nodes, -1, np.int32)
    node_cnt = np.zeros(n_tiles, np.int32)
    edge_sum = np.zeros(n_tiles, np.int64)
    for d in sorted(set(deg.tolist()), reverse=True):
        nodes = np.where(deg == d)[0]
        i = 0
        while i < len(nodes):
            elig = np.where((node_cnt < cap_nodes)
                            & (edge_sum + d <= cap_edges))[0]
            if len(elig) == 0:
                elig = np.where(node_cnt < cap_nodes)[0]
            k = min(len(nodes) - i, len(elig))
            if k < len(elig):
                sel = elig[np.argpartition(edge_sum[elig], k - 1)[:k]]
            else:
                sel = elig
            batch = nodes[i:i + k]
            tile_of[batch] = sel
            col_of[batch] = node_cnt[sel]
            node_cnt[sel] += 1
            edge_sum[sel] += d
            i += k
    assert np.all(tile_of >= 0)
    return tile_of, col_of, int(edge_sum.max())


def _build_streams(src, dst, feat_src16, tile_of, col_of, ch):
    """Bucket edges by packed dst tile; materialize per-core streams.

    Returns (fs, col):
      fs  : [n_cores, NT, P, ch*(D+1) + 2*ch*P] fp16 -- merged stream:
            first ch*(D+1): normal layout [chunk k -> payload | 1.0],
            then 2*ch*P: transposed layout [half h -> feat dsub, k*P+p]
      col : [n_cores, P, NT, ch] f32 -- dst col within tile (999 pad)
    """
    e = len(src)
    tile_e = tile_of[dst]
    col_e = col_of[dst]
    key = tile_e.astype(np.int64) * P + col_e
    order = np.argsort(key, kind="stable")
    t_s = tile_e[order]
    c_s = col_e[order]
    s_s = src[order]
    tstart = np.searchsorted(t_s, np.arange(NT_G))
    pos = np.arange(e) - tstart[t_s]
    assert pos.max() < ch * P
    k = pos // P
    p = pos % P
    core = t_s // NT
    tl = t_s % NT

    f_idx = np.zeros((N_CORES, NT, ch, P), np.int32)
    col = np.full((N_CORES, NT, ch, P), 999.0, np.float32)
    f_idx[core, tl, k, p] = s_s
    col[core, tl, k, p] = c_s.astype(np.float32)

    DP1 = D + 1
    CW = ch * P
    fs = np.zeros((N_CORES, NT, P, ch * DP1 + 2 * CW), np.float16)
    g = feat_src16[f_idx]                       # [c, NT, ch, P, D]
    fnv = fs[..., :ch * DP1].reshape(N_CORES, NT, P, ch, DP1)
    fnv[..., :D] = g.transpose(0, 1, 3, 2, 4)
    fnv[..., D] = 1.0
    gt = g.transpose(0, 1, 4, 2, 3).reshape(N_CORES, NT, 2, P, CW)
    fs[..., ch * DP1:] = gt.transpose(0, 1, 3, 2, 4).reshape(
        N_CORES, NT, P, 2 * CW)
    col = np.ascontiguousarray(col.transpose(0, 3, 1, 2))  # [c, P, NT, ch]
    return fs, col


def _build_gsrcT(feat_dst16, tile_of, col_of):
    """Dst-node features in packed slot order, transposed: [c, 2, P, padr]."""
    padr = NT * P
    out = np.zeros((N_CORES, 2, P, padr), np.float16)
    core = tile_of // NT
    slot = (tile_of % NT).astype(np.int64) * P + col_of
    for c in range(N_CORES):
        m = core == c
        tmp = np.zeros((padr, D), np.float16)
        tmp[slot[m]] = feat_dst16[np.where(m)[0]]
        out[c] = tmp.T.reshape(2, P, padr)
    return out


# --------------------------------------------------------------------------
# Device kernel builder
# --------------------------------------------------------------------------

def build_nc(ch, nt, debug=False, noop=False, reps=1):
    import concourse.bacc as bacc
    import concourse.mybir as mybir
    import concourse.tile as tile

    f16 = mybir.dt.float16
    f32 = mybir.dt.float32
    Alu = mybir.AluOpType
    Act = mybir.ActivationFunctionType

    padr = nt * P
    DP1 = D + 1
    CW = ch * P  # chunk-group width (columns of grouped score PSUM)
    KFIN = next(k for k in (14, 7, 6, 5, 4, 3, 2, 1) if nt % k == 0)
    GB = next(g for g in (7, 4, 2, 1) if nt % g == 0)  # tiles per gT block

    nc = bacc.Bacc("TRN2", target_bir_lowering=False, debug=debug)

    SW = ch * DP1 + 2 * CW  # merged stream width
    gsrcT, wt, fs_s, meta_c = {}, {}, {}, {}
    for et in ("ab", "ba"):
        gsrcT[et] = nc.dram_tensor(f"gsrcT_{et}", [2, P, padr], f16,
                                   kind="ExternalInput")
        wt[et] = nc.dram_tensor(f"wt_{et}", [D, D], f16, kind="ExternalInput")
        fs_s[et] = nc.dram_tensor(f"fs_{et}", [nt, P, SW], f16,
                                  kind="ExternalInput")
        meta_c[et] = nc.dram_tensor(f"mc_{et}", [P, nt, ch], f32,
                                    kind="ExternalInput")
    iota_in = nc.dram_tensor("iota_mat", [P, P], f32, kind="ExternalInput")
    out = nc.dram_tensor("out", [2, padr, D], f16, kind="ExternalOutput")
    out_idx = {"ab": 1, "ba": 0}

    if noop:
        with tile.TileContext(nc) as tc:
            with tc.tile_pool(name="np", bufs=1) as pool:
                z = pool.tile([P, P], f32, tag="z")
                nc.sync.dma_start(z[:], iota_in[:])
                zh = pool.tile([P, P], f16, tag="zh")
                nc.vector.tensor_copy(zh[:], z[:])
                nc.sync.dma_start(out[0, 0:P, 0:P], zh[:])
        nc.compile()
        return nc

    from contextlib import nullcontext

    with tile.TileContext(nc) as tc:
        with (
            tc.tile_pool(name="const", bufs=1) as cpool,
            tc.tile_pool(name="gprep", bufs=2) as gprep,
            tc.tile_pool(name="gps", bufs=1, space="PSUM") as gps,
            tc.tile_pool(name="fs", bufs=2) as fspool,
            tc.tile_pool(name="sp_ps", bufs=2, space="PSUM") as spps,
            tc.tile_pool(name="wj", bufs=6) as wjpool,
            tc.tile_pool(name="small", bufs=16) as spool,
            tc.tile_pool(name="fin", bufs=KFIN + 2) as finpool,
            tc.tile_pool(name="finb", bufs=2) as finbpool,
            tc.tile_pool(name="outp", bufs=2) as outpool,
            tc.tile_pool(name="mm_ps", bufs=2, space="PSUM") as mmps,
            (tc.For_i(0, reps, 1) if reps > 1 else nullcontext()),
        ):
            iota_sb = cpool.tile([P, P], f32, tag="iota")
            nc.sync.dma_start(iota_sb[:], iota_in[:])

            for et in ("ab", "ba"):
                # WT_eff stationary: wtsb[:, r, :] = WT_eff rows r*128..
                wtsb = cpool.tile([P, 2, D], f16, tag=f"wt_{et}",
                                  name=f"wt_{et}")
                nc.sync.dma_start(wtsb[:, 0, :], wt[et][0:P, :])
                nc.sync.dma_start(wtsb[:, 1, :], wt[et][P:D, :])
                mc_sb = cpool.tile([P, nt, ch], f32, tag=f"mc_{et}",
                                   name=f"mc_{et}")
                nc.sync.dma_start(mc_sb[:], meta_c[et][:])

                st = {}
                pend = {}

                def gt_block(tb, et=et, st=st, wtsb=wtsb):
                    """gT window for tiles tb..tb+GB-1 into SBUF (fp16)."""
                    w0 = tb * P
                    ftT = gprep.tile([P, 2, GB * P], f16, tag="ftT",
                                     name="ftT")
                    nc.scalar.dma_start(
                        ftT[:],
                        gsrcT[et][:, :, w0:w0 + GB * P].rearrange(
                            "h p w -> p h w"))
                    gtw = gprep.tile([P, 2, GB * P], f16, tag="gtw",
                                     name="gtw")
                    # matmul out regions must stay <=512 f32 (one PSUM bank)
                    segs = [(s, min(s + 512, GB * P))
                            for s in range(0, GB * P, 512)]
                    for kh in range(2):
                        gp = gps.tile([P, GB * P], f32, tag="gp", name="gp")
                        for s0, s1 in segs:
                            nc.tensor.matmul(
                                gp[:, s0:s1],
                                lhsT=wtsb[:, 0, kh * P:(kh + 1) * P],
                                rhs=ftT[:, 0, s0:s1], start=True, stop=False)
                            nc.tensor.matmul(
                                gp[:, s0:s1],
                                lhsT=wtsb[:, 1, kh * P:(kh + 1) * P],
                                rhs=ftT[:, 1, s0:s1], start=False, stop=True)
                        # Pool can't read PSUM on real HW; ACT copy shares
                        # the exp table (no act-table reload)
                        nc.scalar.copy(gtw[:, kh, :], gp[:])
                    st["gtw"] = gtw

                def stage_a(t, et=et, st=st, pend=pend):
                    """Stream loads + PE scores (grouped PSUM) + exp."""
                    if t % GB == 0:
                        gt_block(t)
                        # one batched stream DMA per GB tiles (DMA issue
                        # overhead dominates; batch to cut instruction count)
                        fS7 = fspool.tile([P, GB, SW], f16, tag="fS7",
                                          name="fS7")
                        nc.sync.dma_start(
                            fS7[:], fs_s[et][t:t + GB].rearrange(
                                "b p w -> p b w"))
                        st["fS7"] = fS7
                    gtw = st["gtw"]
                    go = (t % GB) * P
                    fS = st["fS7"][:, t % GB, :]
                    tb = ch * DP1  # transposed-layout base offset
                    spg = spps.tile([P, CW], f32, tag="spg", name="spg")
                    for k in range(ch):
                        kc = slice(k * P, (k + 1) * P)
                        nc.tensor.matmul(
                            spg[:, kc],
                            lhsT=fS[:, tb + k * P:tb + (k + 1) * P],
                            rhs=gtw[:, 0, go:go + P], start=True, stop=False)
                        nc.tensor.matmul(
                            spg[:, kc],
                            lhsT=fS[:, tb + CW + k * P:tb + CW + (k + 1) * P],
                            rhs=gtw[:, 1, go:go + P], start=False, stop=True)
                    e_t = wjpool.tile([P, CW], f16, tag="e_t", name="e_t")
                    nc.scalar.activation(e_t[:], spg[:], Act.Exp)
                    pend[t] = (fS, e_t)

                def stage_b(t, et=et, st=st, pend=pend):
                    if t % KFIN == 0:
                        st["hsum_b"] = finbpool.tile([P, KFIN], f32,
                                                     tag="hsum_b",
                                                     name="hsum_b")
                        st["hss_b"] = finbpool.tile([P, KFIN], f32,
                                                    tag="hss_b",
                                                    name="hss_b")
                        st["attc_b"] = finbpool.tile([P, KFIN], f32,
                                                     tag="attc_b",
                                                     name="attc_b")
                        st["hs"] = []
                    hsum_b, hss_b = st["hsum_b"], st["hss_b"]
                    attc_b, hs = st["attc_b"], st["hs"]
                    j = t % KFIN
                    fS, e_t = pend.pop(t)

                    # one-hot masks per chunk (Pool), W = E' * mask (DVE)
                    mask = wjpool.tile([P, CW], f16, tag="mask", name="mask")
                    for k in range(ch):
                        nc.gpsimd.tensor_scalar(
                            out=mask[:, k * P:(k + 1) * P], in0=iota_sb[:],
                            scalar1=mc_sb[:, t, k:k + 1], scalar2=None,
                            op0=Alu.is_equal)
                    W = wjpool.tile([P, CW], f16, tag="W", name="W")
                    nc.vector.tensor_tensor(
                        out=W[:], in0=e_t[:], in1=mask[:], op=Alu.mult)

                    acc = mmps.tile([P, DP1], f32, tag="acc", name="acc")
                    for k in range(ch):
                        nc.tensor.matmul(
                            acc[:], lhsT=W[:, k * P:(k + 1) * P],
                            rhs=fS[:, k * DP1:(k + 1) * DP1],
                            start=(k == 0), stop=(k == ch - 1))

                    # un-normalized h_raw = relu(acc); att normalization is
                    # folded into the final scale (relu(x*r)=relu(x)*r for
                    # r>0), removing attc->r1 from the relu critical path
                    # clamp >= 1e-8: r1^2 must not overflow f32 on padded
                    # rows (real rows have attsum = sum(exp) >> 1e-8)
                    nc.vector.tensor_scalar_max(attc_b[:, j:j + 1],
                                                acc[:, D:D + 1], 1e-8)
                    h = finpool.tile([P, D], f16, tag="h", name="h")
                    nc.scalar.activation(h[:], acc[:, 0:D], Act.Relu,
                                         accum_out=hsum_b[:, j:j + 1])
                    hs.append(h)
                    junk = wjpool.tile([P, D], f32, tag="junk", name="junk")
                    nc.scalar.activation(junk[:], h[:], Act.Square,
                                         accum_out=hss_b[:, j:j + 1])

                    if j == KFIN - 1:
                        # batched stats for KFIN tiles (few big ops instead
                        # of 4 small ops per tile)
                        r1_b = finbpool.tile([P, KFIN], f32, tag="r1_b",
                                             name="r1_b")
                        nc.vector.reciprocal(r1_b[:], attc_b[:])
                        # mu_s = hsum_raw * r1 / D ; var = hss*r1^2/D - mu_s^2
                        mu_b = finbpool.tile([P, KFIN], f32, tag="mu_b",
                                             name="mu_b")
                        nc.gpsimd.tensor_tensor(out=mu_b[:], in0=hsum_b[:],
                                                in1=r1_b[:], op=Alu.mult)
                        nc.gpsimd.tensor_scalar_mul(mu_b[:], mu_b[:], 1.0 / D)
                        r2_b = finbpool.tile([P, KFIN], f32, tag="r2_b",
                                             name="r2_b")
                        nc.gpsimd.tensor_tensor(out=r2_b[:], in0=r1_b[:],
                                                in1=r1_b[:], op=Alu.mult)
                        ss_b = finbpool.tile([P, KFIN], f32, tag="ss_b",
                                             name="ss_b")
                        nc.gpsimd.tensor_tensor(out=ss_b[:], in0=hss_b[:],
                                                in1=r2_b[:], op=Alu.mult)
                        mu2_b = finbpool.tile([P, KFIN], f32, tag="mu2_b",
                                              name="mu2_b")
                        nc.gpsimd.tensor_tensor(out=mu2_b[:], in0=mu_b[:],
                                                in1=mu_b[:], op=Alu.mult)
                        nc.gpsimd.tensor_scalar_sub(mu2_b[:], mu2_b[:], EPS)
                        veps_b = finbpool.tile([P, KFIN], f32, tag="veps_b",
                                               name="veps_b")
                        nc.gpsimd.tensor_scalar(
                            out=veps_b[:], in0=ss_b[:], scalar1=1.0 / D,
                            scalar2=None, op0=Alu.mult)
                        nc.gpsimd.tensor_tensor(out=veps_b[:], in0=veps_b[:],
                                                in1=mu2_b[:],
                                                op=Alu.subtract)
                        sd_b = finbpool.tile([P, KFIN], f32, tag="sd_b",
                                             name="sd_b")
                        nc.scalar.sqrt(sd_b[:], veps_b[:])
                        rstd_b = finbpool.tile([P, KFIN], f32, tag="rstd_b",
                                               name="rstd_b")
                        nc.vector.reciprocal(rstd_b[:], sd_b[:])
                        rr_b = finbpool.tile([P, KFIN], f32, tag="rr_b",
                                             name="rr_b")
                        nc.vector.tensor_tensor(
                            out=rr_b[:], in0=r1_b[:], in1=rstd_b[:],
                            op=Alu.mult)
                        posmur_b = finbpool.tile([P, KFIN], f32,
                                                 tag="posmur_b",
                                                 name="posmur_b")
                        nc.vector.tensor_tensor(
                            out=posmur_b[:], in0=mu_b[:], in1=rstd_b[:],
                            op=Alu.mult)
                        o_b = outpool.tile([P, KFIN, D], f16, tag="o_b",
                                           name="o_b")
                        for jj in range(KFIN):
                            nc.vector.tensor_scalar(
                                out=o_b[:, jj, :], in0=hs[jj][:],
                                scalar1=rr_b[:, jj:jj + 1],
                                scalar2=posmur_b[:, jj:jj + 1],
                                op0=Alu.mult, op1=Alu.subtract)
                        rows = slice((t - (KFIN - 1)) * P, (t + 1) * P)
                        dst_ap = out[out_idx[et], rows, :].rearrange(
                            "(q p) d -> p q d", p=P)
                        nc.scalar.dma_start(dst_ap, o_b[:])

                # lag-2 software pipeline; stage_b first so blocked
                # stage_a heads don't HOL-block ready stage_b work in the
                # 4-deep engine wait queues
                LAG = 2
                for t in range(nt + LAG):
                    if t >= LAG:
                        stage_b(t - LAG)
                    if t < nt:
                        stage_a(t)

    nc.compile()
    return nc


# --------------------------------------------------------------------------
# Host orchestration
# --------------------------------------------------------------------------

def _prepare_inputs(feat_a, feat_b, src_ab, dst_ab, src_ba, dst_ba,
                    WT_ab, WT_ba, WA_ab, WA_ba, n_cores, na, nb, ch=None):
    assert n_cores == N_CORES and na == NA and nb == NB
    nt = NT
    feat_a16 = feat_a.astype(np.float16)
    feat_b16 = feat_b.astype(np.float16)

    deg_b = np.bincount(dst_ab, minlength=nb)
    deg_a = np.bincount(dst_ba, minlength=na)
    tile_b, col_b, mx_b = _pack_tiles(deg_b, NT_G)
    tile_a, col_a, mx_a = _pack_tiles(deg_a, NT_G)
    ch_f = max(math.ceil(mx_b / P), math.ceil(mx_a / P))
    if ch is not None:
        assert ch >= ch_f
        ch_f = ch

    fs_ab, c_ab = _build_streams(src_ab, dst_ab, feat_a16,
                                 tile_b, col_b, ch_f)
    fs_ba, c_ba = _build_streams(src_ba, dst_ba, feat_b16,
                                 tile_a, col_a, ch_f)
    gsrcT_ab = _build_gsrcT(feat_b16, tile_b, col_b)  # dst of ab = b nodes
    gsrcT_ba = _build_gsrcT(feat_a16, tile_a, col_a)  # dst of ba = a nodes

    wt_ab = (WA_ab[0][:, None] * WT_ab).astype(np.float16)
    wt_ba = (WA_ba[0][:, None] * WT_ba).astype(np.float16)
    iota_mat = np.broadcast_to(np.arange(P, dtype=np.float32), (P, P)).copy()

    in_maps = []
    for c in range(n_cores):
        in_maps.append({
            "gsrcT_ab": gsrcT_ab[c], "gsrcT_ba": gsrcT_ba[c],
            "wt_ab": wt_ab, "wt_ba": wt_ba,
            "fs_ab": fs_ab[c], "mc_ab": c_ab[c],
            "fs_ba": fs_ba[c], "mc_ba": c_ba[c],
            "iota_mat": iota_mat,
        })
    # slot maps for unsharding: global out row per node
    slot_a = tile_a.astype(np.int64) * P + col_a  # for out[0] (etype ba)
    slot_b = tile_b.astype(np.int64) * P + col_b  # for out[1] (etype ab)
    return in_maps, ch_f, nt, slot_a, slot_b


def kernel(feat_a, feat_b, src_ab, dst_ab, src_ba, dst_ba,
           WT_ab, WT_ba, WA_ab, WA_ba, gamma, beta):
    global LAST_RESULT
    from concourse.bass_utils import run_bass_kernel_spmd

    feat_a = np.asarray(feat_a, np.float32)
    feat_b = np.asarray(feat_b, np.float32)
    src_ab = np.asarray(src_ab, np.int32)
    dst_ab = np.asarray(dst_ab, np.int32)
    src_ba = np.asarray(src_ba, np.int32)
    dst_ba = np.asarray(dst_ba, np.int32)
    WT_ab = np.asarray(WT_ab, np.float32)
    WT_ba = np.asarray(WT_ba, np.float32)
    WA_ab = np.asarray(WA_ab, np.float32)
    WA_ba = np.asarray(WA_ba, np.float32)
    gamma = np.asarray(gamma, np.float32)
    beta = np.asarray(beta, np.float32)

    in_maps, ch, nt, slot_a, slot_b = _prepare_inputs(
        feat_a, feat_b, src_ab, dst_ab, src_ba, dst_ba,
        WT_ab, WT_ba, WA_ab, WA_ba, N_CORES, NA, NB)

    nc = build_nc(ch, nt, debug=False)

    res = run_bass_kernel_spmd(nc, in_maps, list(range(N_CORES)), trace=TRACE)
    LAST_RESULT = res

    dev0 = np.concatenate([res.results[c]["out"][0] for c in range(N_CORES)],
                          axis=0)
    dev1 = np.concatenate([res.results[c]["out"][1] for c in range(N_CORES)],
                          axis=0)
    out = np.empty((2, NA, D), np.float32)
    out[0] = dev0[slot_a].astype(np.float32)
    out[1] = dev1[slot_b].astype(np.float32)

    if not (np.all(gamma == 1.0) and np.all(beta == 0.0)):
        out = out * gamma[None, None, :] + beta[None, None, :]
    return out



# revision 10
# speedup vs baseline: 4.0603x; 4.0603x over previous
"""Trainium2 Bass kernel for nn_RelationalAGG (2-etype GNN attention message passing).

Design: dst-sharded across 8 cores, zero collectives, zero indirect DMA.

 - Host packs dst nodes into (<=128-node, <=640-edge) tiles via balanced
   bin-packing, so every tile needs exactly ch=5 edge chunks (vs 6-7 with
   contiguous range sharding) and all cores carry equal load.
 - Host gathers per-edge src features into two fp16 stream layouts:
   normal [edge, 257] (payload + ones column) for the PE scatter matmul,
   and transposed [feat, edge] for the PE score matmuls. Dst-node feature
   tables are shipped pre-transposed (plain contiguous DMA, no DMA
   transpose on device).
 - Device, per 128-dst tile:
     gT window = WT_eff^T-block matmuls vs pre-transposed dst features
     scores    S' [e, 5*128] = fT-block matmuls into ONE grouped PSUM
               region (all-pairs vs the tile's 128 dst columns)
     E' = exp(S') -- ONE batched ACT op per tile
     W = E'*mask as ONE DVE op (one-hot dst-col masks are precomputed
     on host and shipped fp8 -- on-device Pool mask gen was 2066ns per
     [128,128] chunk = 97% GpSimd occupancy, the old bottleneck)
     PSUM [P, 257] += W_k^T @ [f | 1]  (PE segment-sum: messages+att sums)
   then normalize+relu (ACT, accum_out stats), variance via DVE
   tensor_tensor_reduce, layernorm applied via DVE tensor_scalar, with
   sqrt batched KFIN=14 tiles to amortize ACT table loads. Output fp16.
 - Engine balance (modeled): DMA ~2.2us/tile (bottleneck), ACT ~1.3,
   DVE ~1.1, PE ~1.3, Pool ~0.8.
"""

import math
import sys

import numpy as np

for _p in ("/opt/trn_rl_repo",):
    if _p not in sys.path:
        sys.path.insert(0, _p)

# Problem constants (hardcoded per the task contract).
NA = 100000
NB = 100000
E = 500000
D = 256
EPS = 1e-5
N_CORES = 8
P = 128
NT = 98              # tiles per core per etype
NT_G = NT * N_CORES  # global tiles per etype

TRACE = False
LAST_RESULT = None


# --------------------------------------------------------------------------
# Host-side preprocessing
# --------------------------------------------------------------------------

def _pack_tiles(deg, n_tiles, cap_nodes=P, cap_edges=5 * P):
    """Balanced bin-packing: nodes -> tiles s.t. each tile has <=cap_nodes
    nodes and (usually) <=cap_edges edges. Greedy best-fit by descending
    degree, vectorized per degree class."""
    n_nodes = len(deg)
    tile_of = np.full(n_nodes, -1, np.int32)
    col_of = np.full(n_nodes, -1, np.int32)
    node_cnt = np.zeros(n_tiles, np.int32)
    edge_sum = np.zeros(n_tiles, np.int64)
    for d in sorted(set(deg.tolist()), reverse=True):
        nodes = np.where(deg == d)[0]
        i = 0
        while i < len(nodes):
            elig = np.where((node_cnt < cap_nodes)
                            & (edge_sum + d <= cap_edges))[0]
            if len(elig) == 0:
                elig = np.where(node_cnt < cap_nodes)[0]
            k = min(len(nodes) - i, len(elig))
            if k < len(elig):
                sel = elig[np.argpartition(edge_sum[elig], k - 1)[:k]]
            else:
                sel = elig
            batch = nodes[i:i + k]
            tile_of[batch] = sel
            col_of[batch] = node_cnt[sel]
            node_cnt[sel] += 1
            edge_sum[sel] += d
            i += k
    assert np.all(tile_of >= 0)
    return tile_of, col_of, int(edge_sum.max())


def _build_streams(src, dst, feat_src16, tile_of, col_of, ch):
    """Bucket edges by packed dst tile; materialize per-core streams.

    Returns (fs, mk):
      fs  : [n_cores, NT, P, ch*(D+1) + 2*ch*P] fp16 -- merged stream:
            first ch*(D+1): normal layout [chunk k -> payload | 1.0],
            then 2*ch*P: transposed layout [half h -> feat dsub, k*P+p]
      mk  : [n_cores, NT, P, ch*P] fp8 -- one-hot dst-col masks:
            mk[.., p, k*P + c] = 1 iff edge (k,p) has dst col c
    """
    import ml_dtypes

    e = len(src)
    tile_e = tile_of[dst]
    col_e = col_of[dst]
    key = tile_e.astype(np.int64) * P + col_e
    order = np.argsort(key, kind="stable")
    t_s = tile_e[order]
    c_s = col_e[order]
    s_s = src[order]
    tstart = np.searchsorted(t_s, np.arange(NT_G))
    pos = np.arange(e) - tstart[t_s]
    assert pos.max() < ch * P
    k = pos // P
    p = pos % P
    core = t_s // NT
    tl = t_s % NT

    CW = ch * P
    f_idx = np.zeros((N_CORES, NT, ch, P), np.int32)
    f_idx[core, tl, k, p] = s_s
    mk = np.zeros((N_CORES, NT, P, CW), ml_dtypes.float8_e4m3)
    mk[core, tl, p, k * P + c_s] = 1.0

    DP1 = D + 1
    fs = np.zeros((N_CORES, NT, P, ch * DP1 + 2 * CW), np.float16)
    g = feat_src16[f_idx]                       # [c, NT, ch, P, D]
    fnv = fs[..., :ch * DP1].reshape(N_CORES, NT, P, ch, DP1)
    fnv[..., :D] = g.transpose(0, 1, 3, 2, 4)
    fnv[..., D] = 1.0
    gt = g.transpose(0, 1, 4, 2, 3).reshape(N_CORES, NT, 2, P, CW)
    fs[..., ch * DP1:] = gt.transpose(0, 1, 3, 2, 4).reshape(
        N_CORES, NT, P, 2 * CW)
    return fs, mk


def _build_gsrcT(feat_dst16, tile_of, col_of):
    """Dst-node features in packed slot order, transposed: [c, 2, P, padr]."""
    padr = NT * P
    out = np.zeros((N_CORES, 2, P, padr), np.float16)
    core = tile_of // NT
    slot = (tile_of % NT).astype(np.int64) * P + col_of
    for c in range(N_CORES):
        m = core == c
        tmp = np.zeros((padr, D), np.float16)
        tmp[slot[m]] = feat_dst16[np.where(m)[0]]
        out[c] = tmp.T.reshape(2, P, padr)
    return out


# --------------------------------------------------------------------------
# Device kernel builder
# --------------------------------------------------------------------------

def build_nc(ch, nt, debug=False, noop=False, reps=1):
    import concourse.bacc as bacc
    import concourse.mybir as mybir
    import concourse.tile as tile

    f16 = mybir.dt.float16
    f32 = mybir.dt.float32
    f8 = mybir.dt.float8e4
    Alu = mybir.AluOpType
    Act = mybir.ActivationFunctionType

    padr = nt * P
    DP1 = D + 1
    CW = ch * P  # chunk-group width (columns of grouped score PSUM)
    KFIN = next(k for k in (14, 7, 6, 5, 4, 3, 2, 1) if nt % k == 0)
    GB = next(g for g in (7, 4, 2, 1) if nt % g == 0)  # tiles per gT block

    nc = bacc.Bacc("TRN2", target_bir_lowering=False, debug=debug)

    SW = ch * DP1 + 2 * CW  # merged stream width
    gsrcT, wt, fs_s, mk_s = {}, {}, {}, {}
    for et in ("ab", "ba"):
        gsrcT[et] = nc.dram_tensor(f"gsrcT_{et}", [2, P, padr], f16,
                                   kind="ExternalInput")
        wt[et] = nc.dram_tensor(f"wt_{et}", [D, D], f16, kind="ExternalInput")
        fs_s[et] = nc.dram_tensor(f"fs_{et}", [nt, P, SW], f16,
                                  kind="ExternalInput")
        mk_s[et] = nc.dram_tensor(f"mk_{et}", [nt, P, CW], f8,
                                  kind="ExternalInput")
    out = nc.dram_tensor("out", [2, padr, D], f16, kind="ExternalOutput")
    out_idx = {"ab": 1, "ba": 0}

    if noop:
        with tile.TileContext(nc) as tc:
            with tc.tile_pool(name="np", bufs=1) as pool:
                z = pool.tile([P, P], f16, tag="z")
                nc.sync.dma_start(z[:], wt["ab"][0:P, 0:P])
                zh = pool.tile([P, P], f16, tag="zh")
                nc.vector.tensor_copy(zh[:], z[:])
                nc.sync.dma_start(out[0, 0:P, 0:P], zh[:])
        nc.compile()
        return nc

    from contextlib import nullcontext

    with tile.TileContext(nc) as tc:
        with (
            tc.tile_pool(name="const", bufs=1) as cpool,
            tc.tile_pool(name="gprep", bufs=2) as gprep,
            tc.tile_pool(name="gps", bufs=1, space="PSUM") as gps,
            tc.tile_pool(name="fs", bufs=2) as fspool,
            tc.tile_pool(name="sp_ps", bufs=2, space="PSUM") as spps,
            tc.tile_pool(name="wj", bufs=6) as wjpool,
            tc.tile_pool(name="small", bufs=16) as spool,
            tc.tile_pool(name="fin", bufs=KFIN + 2) as finpool,
            tc.tile_pool(name="finb", bufs=2) as finbpool,
            tc.tile_pool(name="outp", bufs=2) as outpool,
            tc.tile_pool(name="mm_ps", bufs=2, space="PSUM") as mmps,
            (tc.For_i(0, reps, 1) if reps > 1 else nullcontext()),
        ):
            for et in ("ab", "ba"):
                # WT_eff stationary: wtsb[:, r, :] = WT_eff rows r*128..
                wtsb = cpool.tile([P, 2, D], f16, tag=f"wt_{et}",
                                  name=f"wt_{et}")
                nc.sync.dma_start(wtsb[:, 0, :], wt[et][0:P, :])
                nc.sync.dma_start(wtsb[:, 1, :], wt[et][P:D, :])

                st = {}
                pend = {}

                def gt_block(tb, et=et, st=st, wtsb=wtsb):
                    """gT window for tiles tb..tb+GB-1 into SBUF (fp16)."""
                    w0 = tb * P
                    ftT = gprep.tile([P, 2, GB * P], f16, tag="ftT",
                                     name="ftT")
                    nc.scalar.dma_start(
                        ftT[:],
                        gsrcT[et][:, :, w0:w0 + GB * P].rearrange(
                            "h p w -> p h w"))
                    gtw = gprep.tile([P, 2, GB * P], f16, tag="gtw",
                                     name="gtw")
                    # matmul out regions must stay <=512 f32 (one PSUM bank)
                    segs = [(s, min(s + 512, GB * P))
                            for s in range(0, GB * P, 512)]
                    for kh in range(2):
                        gp = gps.tile([P, GB * P], f32, tag="gp", name="gp")
                        for s0, s1 in segs:
                            nc.tensor.matmul(
                                gp[:, s0:s1],
                                lhsT=wtsb[:, 0, kh * P:(kh + 1) * P],
                                rhs=ftT[:, 0, s0:s1], start=True, stop=False)
                            nc.tensor.matmul(
                                gp[:, s0:s1],
                                lhsT=wtsb[:, 1, kh * P:(kh + 1) * P],
                                rhs=ftT[:, 1, s0:s1], start=False, stop=True)
                        # Pool can't read PSUM on real HW; ACT copy shares
                        # the exp table (no act-table reload)
                        nc.scalar.copy(gtw[:, kh, :], gp[:])
                    st["gtw"] = gtw

                def stage_a(t, et=et, st=st, pend=pend):
                    """Stream loads + PE scores (grouped PSUM) + exp."""
                    if t % GB == 0:
                        gt_block(t)
                        # one batched stream DMA per GB tiles (DMA issue
                        # overhead dominates; batch to cut instruction count)
                        fS7 = fspool.tile([P, GB, SW], f16, tag="fS7",
                                          name="fS7")
                        nc.sync.dma_start(
                            fS7[:], fs_s[et][t:t + GB].rearrange(
                                "b p w -> p b w"))
                        st["fS7"] = fS7
                        mk7 = fspool.tile([P, GB, CW], f8, tag="mk7",
                                          name="mk7")
                        nc.sync.dma_start(
                            mk7[:], mk_s[et][t:t + GB].rearrange(
                                "b p w -> p b w"))
                        st["mk7"] = mk7
                    gtw = st["gtw"]
                    go = (t % GB) * P
                    fS = st["fS7"][:, t % GB, :]
                    tb = ch * DP1  # transposed-layout base offset
                    spg = spps.tile([P, CW], f32, tag="spg", name="spg")
                    for k in range(ch):
                        kc = slice(k * P, (k + 1) * P)
                        nc.tensor.matmul(
                            spg[:, kc],
                            lhsT=fS[:, tb + k * P:tb + (k + 1) * P],
                            rhs=gtw[:, 0, go:go + P], start=True, stop=False)
                        nc.tensor.matmul(
                            spg[:, kc],
                            lhsT=fS[:, tb + CW + k * P:tb + CW + (k + 1) * P],
                            rhs=gtw[:, 1, go:go + P], start=False, stop=True)
                    e_t = wjpool.tile([P, CW], f16, tag="e_t", name="e_t")
                    nc.scalar.activation(e_t[:], spg[:], Act.Exp)
                    pend[t] = (fS, e_t, st["mk7"])

                def stage_b(t, et=et, st=st, pend=pend):
                    if t % KFIN == 0:
                        st["hsum_b"] = finbpool.tile([P, KFIN], f32,
                                                     tag="hsum_b",
                                                     name="hsum_b")
                        st["hss_b"] = finbpool.tile([P, KFIN], f32,
                                                    tag="hss_b",
                                                    name="hss_b")
                        st["attc_b"] = finbpool.tile([P, KFIN], f32,
                                                     tag="attc_b",
                                                     name="attc_b")
                        st["hs"] = []
                    hsum_b, hss_b = st["hsum_b"], st["hss_b"]
                    attc_b, hs = st["attc_b"], st["hs"]
                    j = t % KFIN
                    fS, e_t, mk7 = pend.pop(t)

                    # W = E' * host-shipped one-hot mask (DVE, fp8 in1)
                    W = wjpool.tile([P, CW], f16, tag="W", name="W")
                    nc.vector.tensor_tensor(
                        out=W[:], in0=e_t[:], in1=mk7[:, t % GB, :],
                        op=Alu.mult)

                    acc = mmps.tile([P, DP1], f32, tag="acc", name="acc")
                    for k in range(ch):
                        nc.tensor.matmul(
                            acc[:], lhsT=W[:, k * P:(k + 1) * P],
                            rhs=fS[:, k * DP1:(k + 1) * DP1],
                            start=(k == 0), stop=(k == ch - 1))

                    # un-normalized h_raw = relu(acc); att normalization is
                    # folded into the final scale (relu(x*r)=relu(x)*r for
                    # r>0), removing attc->r1 from the relu critical path
                    # clamp >= 1e-8: r1^2 must not overflow f32 on padded
                    # rows (real rows have attsum = sum(exp) >> 1e-8)
                    nc.vector.tensor_scalar_max(attc_b[:, j:j + 1],
                                                acc[:, D:D + 1], 1e-8)
                    h = finpool.tile([P, D], f16, tag="h", name="h")
                    nc.scalar.activation(h[:], acc[:, 0:D], Act.Relu,
                                         accum_out=hsum_b[:, j:j + 1])
                    hs.append(h)
                    junk = wjpool.tile([P, D], f32, tag="junk", name="junk")
                    nc.scalar.activation(junk[:], h[:], Act.Square,
                                         accum_out=hss_b[:, j:j + 1])

                    if j == KFIN - 1:
                        # batched stats for KFIN tiles (few big ops instead
                        # of 4 small ops per tile)
                        r1_b = finbpool.tile([P, KFIN], f32, tag="r1_b",
                                             name="r1_b")
                        nc.vector.reciprocal(r1_b[:], attc_b[:])
                        # mu_s = hsum_raw * r1 / D ; var = hss*r1^2/D - mu_s^2
                        mu_b = finbpool.tile([P, KFIN], f32, tag="mu_b",
                                             name="mu_b")
                        nc.gpsimd.tensor_tensor(out=mu_b[:], in0=hsum_b[:],
                                                in1=r1_b[:], op=Alu.mult)
                        nc.gpsimd.tensor_scalar_mul(mu_b[:], mu_b[:], 1.0 / D)
                        r2_b = finbpool.tile([P, KFIN], f32, tag="r2_b",
                                             name="r2_b")
                        nc.gpsimd.tensor_tensor(out=r2_b[:], in0=r1_b[:],
                                                in1=r1_b[:], op=Alu.mult)
                        ss_b = finbpool.tile([P, KFIN], f32, tag="ss_b",
                                             name="ss_b")
                        nc.gpsimd.tensor_tensor(out=ss_b[:], in0=hss_b[:],
                                                in1=r2_b[:], op=Alu.mult)
                        mu2_b = finbpool.tile([P, KFIN], f32, tag="mu2_b",
                                              name="mu2_b")
                        nc.gpsimd.tensor_tensor(out=mu2_b[:], in0=mu_b[:],
                                                in1=mu_b[:], op=Alu.mult)
                        nc.gpsimd.tensor_scalar_sub(mu2_b[:], mu2_b[:], EPS)
                        veps_b = finbpool.tile([P, KFIN], f32, tag="veps_b",
                                               name="veps_b")
                        nc.gpsimd.tensor_scalar(
                            out=veps_b[:], in0=ss_b[:], scalar1=1.0 / D,
                            scalar2=None, op0=Alu.mult)
                        nc.gpsimd.tensor_tensor(out=veps_b[:], in0=veps_b[:],
                                                in1=mu2_b[:],
                                                op=Alu.subtract)
                        sd_b = finbpool.tile([P, KFIN], f32, tag="sd_b",
                                             name="sd_b")
                        nc.scalar.sqrt(sd_b[:], veps_b[:])
                        rstd_b = finbpool.tile([P, KFIN], f32, tag="rstd_b",
                                               name="rstd_b")
                        nc.vector.reciprocal(rstd_b[:], sd_b[:])
                        rr_b = finbpool.tile([P, KFIN], f32, tag="rr_b",
                                             name="rr_b")
                        nc.vector.tensor_tensor(
                            out=rr_b[:], in0=r1_b[:], in1=rstd_b[:],
                            op=Alu.mult)
                        posmur_b = finbpool.tile([P, KFIN], f32,
                                                 tag="posmur_b",
                                                 name="posmur_b")
                        nc.vector.tensor_tensor(
                            out=posmur_b[:], in0=mu_b[:], in1=rstd_b[:],
                            op=Alu.mult)
                        o_b = outpool.tile([P, KFIN, D], f16, tag="o_b",
                                           name="o_b")
                        for jj in range(KFIN):
                            nc.vector.tensor_scalar(
                                out=o_b[:, jj, :], in0=hs[jj][:],
                                scalar1=rr_b[:, jj:jj + 1],
                                scalar2=posmur_b[:, jj:jj + 1],
                                op0=Alu.mult, op1=Alu.subtract)
                        rows = slice((t - (KFIN - 1)) * P, (t + 1) * P)
                        dst_ap = out[out_idx[et], rows, :].rearrange(
                            "(q p) d -> p q d", p=P)
                        nc.scalar.dma_start(dst_ap, o_b[:])

                # lag-2 software pipeline; stage_b first so blocked
                # stage_a heads don't HOL-block ready stage_b work in the
                # 4-deep engine wait queues
                LAG = 2
                for t in range(nt + LAG):
                    if t >= LAG:
                        stage_b(t - LAG)
                    if t < nt:
                        stage_a(t)

    nc.compile()
    return nc


# --------------------------------------------------------------------------
# Host orchestration
# --------------------------------------------------------------------------

def _prepare_inputs(feat_a, feat_b, src_ab, dst_ab, src_ba, dst_ba,
                    WT_ab, WT_ba, WA_ab, WA_ba, n_cores, na, nb, ch=None):
    assert n_cores == N_CORES and na == NA and nb == NB
    nt = NT
    feat_a16 = feat_a.astype(np.float16)
    feat_b16 = feat_b.astype(np.float16)

    deg_b = np.bincount(dst_ab, minlength=nb)
    deg_a = np.bincount(dst_ba, minlength=na)
    tile_b, col_b, mx_b = _pack_tiles(deg_b, NT_G)
    tile_a, col_a, mx_a = _pack_tiles(deg_a, NT_G)
    ch_f = max(math.ceil(mx_b / P), math.ceil(mx_a / P))
    if ch is not None:
        assert ch >= ch_f
        ch_f = ch

    fs_ab, mk_ab = _build_streams(src_ab, dst_ab, feat_a16,
                                  tile_b, col_b, ch_f)
    fs_ba, mk_ba = _build_streams(src_ba, dst_ba, feat_b16,
                                  tile_a, col_a, ch_f)
    gsrcT_ab = _build_gsrcT(feat_b16, tile_b, col_b)  # dst of ab = b nodes
    gsrcT_ba = _build_gsrcT(feat_a16, tile_a, col_a)  # dst of ba = a nodes

    wt_ab = (WA_ab[0][:, None] * WT_ab).astype(np.float16)
    wt_ba = (WA_ba[0][:, None] * WT_ba).astype(np.float16)

    in_maps = []
    for c in range(n_cores):
        in_maps.append({
            "gsrcT_ab": gsrcT_ab[c], "gsrcT_ba": gsrcT_ba[c],
            "wt_ab": wt_ab, "wt_ba": wt_ba,
            "fs_ab": fs_ab[c], "mk_ab": mk_ab[c],
            "fs_ba": fs_ba[c], "mk_ba": mk_ba[c],
        })
    # slot maps for unsharding: global out row per node
    slot_a = tile_a.astype(np.int64) * P + col_a  # for out[0] (etype ba)
    slot_b = tile_b.astype(np.int64) * P + col_b  # for out[1] (etype ab)
    return in_maps, ch_f, nt, slot_a, slot_b


def kernel(feat_a, feat_b, src_ab, dst_ab, src_ba, dst_ba,
           WT_ab, WT_ba, WA_ab, WA_ba, gamma, beta):
    global LAST_RESULT
    from concourse.bass_utils import run_bass_kernel_spmd

    feat_a = np.asarray(feat_a, np.float32)
    feat_b = np.asarray(feat_b, np.float32)
    src_ab = np.asarray(src_ab, np.int32)
    dst_ab = np.asarray(dst_ab, np.int32)
    src_ba = np.asarray(src_ba, np.int32)
    dst_ba = np.asarray(dst_ba, np.int32)
    WT_ab = np.asarray(WT_ab, np.float32)
    WT_ba = np.asarray(WT_ba, np.float32)
    WA_ab = np.asarray(WA_ab, np.float32)
    WA_ba = np.asarray(WA_ba, np.float32)
    gamma = np.asarray(gamma, np.float32)
    beta = np.asarray(beta, np.float32)

    in_maps, ch, nt, slot_a, slot_b = _prepare_inputs(
        feat_a, feat_b, src_ab, dst_ab, src_ba, dst_ba,
        WT_ab, WT_ba, WA_ab, WA_ba, N_CORES, NA, NB)

    nc = build_nc(ch, nt, debug=False)

    res = run_bass_kernel_spmd(nc, in_maps, list(range(N_CORES)), trace=TRACE)
    LAST_RESULT = res

    dev0 = np.concatenate([res.results[c]["out"][0] for c in range(N_CORES)],
                          axis=0)
    dev1 = np.concatenate([res.results[c]["out"][1] for c in range(N_CORES)],
                          axis=0)
    out = np.empty((2, NA, D), np.float32)
    out[0] = dev0[slot_a].astype(np.float32)
    out[1] = dev1[slot_b].astype(np.float32)

    if not (np.all(gamma == 1.0) and np.all(beta == 0.0)):
        out = out * gamma[None, None, :] + beta[None, None, :]
    return out

